# revision 1
# baseline (speedup 1.0000x reference)
"""Class-conditional label-smoothing cross-entropy loss on 8 Trainium2 cores.

Reference math (C=1000 classes, B=65536 samples, smoothing s=0.1):
    A = softmax(class_avg, axis=-1)                         # [C, C]
    S[t, j] = s * (1 - A[t, j]) / (1 - A[t, t])  (j != t);  S[t, t] = 1 - s
    R[t]    = sum_j S[t, j]
    loss_i  = lse_i * R[t_i] - S[t_i] . x_i,   lse_i = log(sum_j exp(x_ij))
    out     = mean_i loss_i

Data-parallel: x and target are sharded along batch across the 8 cores,
class_avg is replicated. Each core:
  1. builds the smoothing table in its DRAM once:
     tab[t] = [S[t, :] as bf16 (1000) | R[t] as f32 bit-packed in 2 bf16
     slots | zero pad to 1024]   (bf16 halves the per-sample gather traffic;
     since E[x]=0 the S quantization does not bias the mean loss, and R stays
     exact f32 via the bit-pack)
  2. processes 64 tiles of 128 samples (sample p*64+j -> tile j, partition p):
     x tile DMA, indirect-DMA row gather of tab by target, ACT exp with
     accumulate -> sumexp, one fused DVE multiply with accumulate -> dot
  3. tail: lse = ln(sumexp), loss = R*lse - dot, one [128, 64] store.
Host sums the 8 partial grids in f64 and divides by B.
"""

import numpy as np

import concourse.bass as bass
import concourse.tile as tile
from concourse import bacc, mybir
from concourse.bass_utils import run_bass_kernel_spmd

B = 65536
C = 1000
NCORES = 8
BLOC = B // NCORES          # 8192 samples per core
P = 128
NT = BLOC // P              # 64 sample tiles per core
TABW = 1024                 # table row: 1000 bf16 S + f32 R (2 slots) + pad
SM = 0.1

_CACHE = {}


def build_program(reps=1, tab_dt="fp8", abl=(), x_chunk=2, g_chunk=1):
    # abl: timing-ablation switches ("gather" | "x" | "act" | "dve"), each
    # drops that component from the main loop (breaks numerics, timing only).
    # x_chunk: sample tiles per x DMA (2 -> 1 MiB transfers; HW-measured
    # faster than 1 or 8).
    # g_chunk: table rows gathered per indirect DMA per partition. KEEP AT 1:
    # multi-row indirect DMA (3D dest AP) silently corrupts and can hard-wedge
    # the device (NRT_EXEC_UNIT_UNRECOVERABLE) even though CoreSim accepts it.
    # reps>1 repeats the main loop body (same data) for slope-timing in
    # test.py: device time scales with reps, dispatch overhead does not.
    f32 = mybir.dt.float32
    bf16 = mybir.dt.bfloat16
    i32 = mybir.dt.int32
    Alu = mybir.AluOpType
    Act = mybir.ActivationFunctionType
    tdt = {"fp8": mybir.dt.float8e4, "bf16": bf16}[tab_dt]
    # R occupies 4 bytes (bf16 hi/lo pair) right after the C S-entries
    rslots = 4 // mybir.dt.size(tdt)

    nc = bacc.Bacc("TRN2", target_bir_lowering=False, debug=False)
    x_ap = nc.dram_tensor("x", [BLOC, C], f32, kind="ExternalInput").ap()
    ca_ap = nc.dram_tensor("ca", [C, C], f32, kind="ExternalInput").ap()
    tg_ap = nc.dram_tensor("tg", [BLOC], i32, kind="ExternalInput").ap()
    out_ap = nc.dram_tensor("out", [P, NT], f32, kind="ExternalOutput").ap()
    tab_ap = nc.dram_tensor("tab", [C, TABW], tdt).ap()

    with tile.TileContext(nc) as tc:
        with (
            tc.tile_pool(name="tabp", bufs=2) as tabp,
            tc.tile_pool(name="small", bufs=2) as small,
            tc.tile_pool(name="xs", bufs=2) as xs,
            tc.tile_pool(name="gs", bufs=3) as gs,
            tc.tile_pool(name="scr", bufs=2) as scr,
            tc.tile_pool(name="cols", bufs=1) as cols,
        ):
            # target indices: idx[p, j] = tg[p*NT + j]
            idx = cols.tile([P, NT], i32)
            nc.sync.dma_start(idx[:], tg_ap.rearrange("(p c) -> p c", c=NT))

            # ---- smoothing table -------------------------------------------
            for k in range((C + P - 1) // P):
                r0 = k * P
                pr = min(r0 + P, C) - r0
                cat = tabp.tile([P, C], f32, tag="cat")
                nc.sync.dma_start(cat[:pr], ca_ap[r0 : r0 + pr, :])
                e = tabp.tile([P, C], f32, tag="e")
                sume = small.tile([P, 1], f32, tag="sume")
                nc.scalar.activation(e[:pr], cat[:pr], Act.Exp, accum_out=sume[:pr])
                # diagonal e[t, t] via affine mask + row reduce
                msk = tabp.tile([P, C], f32, tag="msk")
                nc.gpsimd.affine_select(
                    out=msk[:pr], in_=e[:pr], compare_op=Alu.is_equal, fill=0.0,
                    base=-r0, channel_multiplier=-1, pattern=[[1, C]],
                )
                ett = small.tile([P, 1], f32, tag="ett")
                nc.vector.tensor_reduce(
                    out=ett[:pr], in_=msk[:pr], axis=mybir.AxisListType.X, op=Alu.add
                )
                den = small.tile([P, 1], f32, tag="den")
                nc.vector.tensor_tensor(
                    out=den[:pr], in0=sume[:pr], in1=ett[:pr], op=Alu.subtract
                )
                rec = small.tile([P, 1], f32, tag="rec")
                nc.vector.reciprocal(rec[:pr], den[:pr])
                negw = small.tile([P, 1], f32, tag="negw")
                nc.vector.tensor_scalar_mul(negw[:pr], rec[:pr], -SM)
                # S_pre[t, j] = (e - sume) * (-s / den); its diagonal equals s,
                # and sum_j S_pre = R - (1 - 2s)
                spre = tabp.tile([P, C], f32, tag="spre")
                rpre = small.tile([P, 1], f32, tag="rpre")
                nc.vector.scalar_tensor_tensor(
                    out=spre[:pr], in0=e[:pr], scalar=sume[:pr],
                    in1=negw[:pr].to_broadcast([pr, C]),
                    op0=Alu.subtract, op1=Alu.mult, accum_out=rpre[:pr],
                )
                sb = tabp.tile([P, TABW], tdt, tag="sb")
                nc.gpsimd.affine_select(
                    out=sb[:pr, 0:C], in_=spre[:pr], compare_op=Alu.not_equal,
                    fill=1.0 - SM, base=-r0, channel_multiplier=-1, pattern=[[1, C]],
                )
                # R as a hi/lo bf16 pair (R = hi + lo, error ~2^-18 relative),
                # bit-packed into the table row right after the S entries
                rt = small.tile([P, 1], f32, tag="rt")
                nc.vector.tensor_scalar_add(rt[:pr], rpre[:pr], 1.0 - 2 * SM)
                rv = sb[:pr, C : C + rslots].bitcast(bf16)
                nc.vector.tensor_copy(out=rv[:, 0:1], in_=rt[:pr])
                nc.vector.tensor_tensor(
                    out=rv[:, 1:2], in0=rt[:pr], in1=rv[:, 0:1], op=Alu.subtract
                )
                nc.vector.memset(sb[:pr, C + rslots : TABW], 0.0)
                nc.sync.dma_start(tab_ap[r0 : r0 + pr, :], sb[:pr])

            # ---- main loop -------------------------------------------------
            # x viewed [P, NT*C]: partition p holds samples p*NT..p*NT+NT-1
            # back to back, so an x_chunk load is one contiguous 2D DMA
            x_r = x_ap.rearrange("(p c) d -> p c d", c=NT)
            x_f = x_ap.rearrange("(p c) d -> p (c d)", c=NT)
            se_cols = cols.tile([P, NT], f32)
            dot_cols = cols.tile([P, NT], f32)
            r_cols = cols.tile([P, NT], f32)
            if abl:
                for t in (se_cols, dot_cols, r_cols):
                    nc.vector.memset(t[:], 1.0)
            xt0 = gt0 = None
            if "x" in abl:
                xt0 = cols.tile([P, C], f32)
                nc.sync.dma_start(xt0[:], x_r[:, 0, :])
            if "gather" in abl:
                gt0 = cols.tile([P, TABW], tdt)
                nc.vector.memset(gt0[:], 0.25)
            xbig = gbig = None
            for j in range(NT * reps):
                j = j % NT
                if "x" in abl:
                    xt = xt0
                else:
                    if j % x_chunk == 0:
                        xbig = xs.tile([P, x_chunk * C], f32)
                        nc.sync.dma_start(
                            xbig[:], x_f[:, j * C : (j + x_chunk) * C]
                        )
                    xt = xbig[:, (j % x_chunk) * C : (j % x_chunk + 1) * C]
                if "gather" in abl:
                    gt = gt0
                elif g_chunk == 1:
                    gt = gs.tile([P, TABW], tdt, tag="gbig")
                    nc.gpsimd.indirect_dma_start(
                        out=gt[:], out_offset=None, in_=tab_ap[:],
                        in_offset=bass.IndirectOffsetOnAxis(ap=idx[:, j : j + 1], axis=0),
                    )
                else:
                    if j % g_chunk == 0:
                        gbig = gs.tile([P, g_chunk, TABW], tdt, tag="gbig")
                        nc.gpsimd.indirect_dma_start(
                            out=gbig[:], out_offset=None, in_=tab_ap[:],
                            in_offset=bass.IndirectOffsetOnAxis(
                                ap=idx[:, j : j + g_chunk], axis=0
                            ),
                        )
                    gt = gbig[:, j % g_chunk, :]
                if "act" not in abl:
                    es = scr.tile([P, C], bf16, tag="es")
                    nc.scalar.activation(
                        es[:], xt[:], Act.Exp, accum_out=se_cols[:, j : j + 1]
                    )
                if "dve" not in abl:
                    ps = scr.tile([P, C], f32, tag="ps")
                    nc.vector.scalar_tensor_tensor(
                        out=ps[:], in0=xt[:], scalar=1.0, in1=gt[:, 0:C],
                        op0=Alu.mult, op1=Alu.mult, accum_out=dot_cols[:, j : j + 1],
                    )
                    grv = gt[:, C : C + rslots].bitcast(bf16)
                    nc.vector.tensor_tensor(
                        out=r_cols[:, j : j + 1], in0=grv[:, 0:1],
                        in1=grv[:, 1:2], op=Alu.add,
                    )

            # ---- tail ------------------------------------------------------
            lse = cols.tile([P, NT], f32)
            nc.scalar.activation(lse[:], se_cols[:], Act.Ln)
            t1 = cols.tile([P, NT], f32)
            nc.vector.tensor_mul(t1[:], r_cols[:], lse[:])
            loss = cols.tile([P, NT], f32)
            nc.vector.tensor_tensor(
                out=loss[:], in0=t1[:], in1=dot_cols[:], op=Alu.subtract
            )
            nc.sync.dma_start(out_ap[:], loss[:])

    nc.compile()
    nc.finalize()
    return nc


def get_program():
    if "nc" not in _CACHE:
        _CACHE["nc"] = build_program()
    return _CACHE["nc"]


def make_in_maps(x, class_avg, target):
    x = np.ascontiguousarray(np.asarray(x, dtype=np.float32))
    ca = np.ascontiguousarray(np.asarray(class_avg, dtype=np.float32))
    tg = np.ascontiguousarray(np.asarray(target).astype(np.int32))
    assert x.shape == (B, C) and ca.shape == (C, C) and tg.shape == (B,)
    return [
        {"x": x[c * BLOC : (c + 1) * BLOC], "ca": ca, "tg": tg[c * BLOC : (c + 1) * BLOC]}
        for c in range(NCORES)
    ]


def reduce_outputs(results):
    tot = 0.0
    for c in range(NCORES):
        tot += results[c]["out"].astype(np.float64).sum()
    return np.array(tot / B, dtype=np.float32)


def kernel(x, class_avg, target):
    nc = get_program()
    in_maps = make_in_maps(x, class_avg, target)
    res = run_bass_kernel_spmd(nc, in_maps, list(range(NCORES)))
    return reduce_outputs(res.results)



# revision 26
# speedup vs baseline: 1.3089x; 1.3089x over previous
"""Class-conditional label-smoothing cross-entropy loss on 8 Trainium2 cores.

Reference math (C=1000 classes, B=65536 samples, smoothing s=0.1):
    A = softmax(class_avg, axis=-1)                         # [C, C]
    S[t, j] = s * (1 - A[t, j]) / (1 - A[t, t])  (j != t);  S[t, t] = 1 - s
    R[t]    = sum_j S[t, j]
    loss_i  = lse_i * R[t_i] - S[t_i] . x_i,   lse_i = log(sum_j exp(x_ij))
    out     = mean_i loss_i

Data-parallel: x and target are sharded along batch across the 8 cores,
class_avg is replicated. Each core:
  1. builds the smoothing table in its DRAM once:
     tab[t] = [S[t, :] as fp8 (1000 B) | R[t] as f32 bit-packed in 2 bf16
     slots | zero pad to 1024 B]   (fp8 quarters the per-sample gather
     traffic; since E[x]=0 the S quantization does not bias the mean loss,
     and R stays exact f32 via the bit-pack)
  2. processes 64 tiles of 128 samples (sample p*64+j -> tile j, partition p):
     x tile DMA; table-row gather via InstDMAGatherAnt (gather_mode="ant",
     512 rows per op, 16 ops/pass -> ~1.2us of Pool SWDGE per op instead of
     ~1us per TILE with indirect_dma_start, and gather issue is decoupled
     from DVE-dot consumption by keeping a full pass of gathered rows in
     SBUF); ACT exp with accumulate -> sumexp; one fused DVE multiply with
     accumulate -> dot
  3. tail: lse = ln(sumexp), loss = R*lse - dot, one [128, 64] store.
Host sums the 8 partial grids in f64 and divides by B.

The gather indices are pre-permuted on the host into the DMAGatherAnt
wrapped-int16 layout ("tgw" input, [128, BLOC/16]) so gathered row i of op k
lands at partition i%128, block i//128 = exactly tile k*4 + i//128 of the
existing x layout (sample p*64+j at partition p).
"""

import numpy as np

import concourse.bass as bass
import concourse.tile as tile
from concourse import bacc, mybir
from concourse.bass_utils import run_bass_kernel_spmd

B = 65536
C = 1000
NCORES = 8
BLOC = B // NCORES          # 8192 samples per core
P = 128
NT = BLOC // P              # 64 sample tiles per core
TABW = 1024                 # table row: 1000 fp8 S + f32 R (2 bf16 slots) + pad
SM = 0.1

_CACHE = {}


def build_program(reps=1, tab_dt="fp8", abl=(), x_chunk=2, g_chunk=1,
                  xs_bufs=6, gs_bufs=None, scr_bufs=2, tabp_bufs=3, tabw=TABW,
                  swdge_scratch=16384, swdge_queues=1,
                  gather_mode="ant", grows=512, pre_x=0, ca_pre=True):
    # abl: timing-ablation switches ("gather" | "x" | "act" | "dve"), each
    # drops that component from the main loop (breaks numerics, timing only).
    # x_chunk: sample tiles per x DMA (2 -> 1 MiB transfers; HW-measured
    # faster than 1 or 8).
    # gather_mode:
    #   "ant": InstDMAGatherAnt, `grows` table rows per op (994ns + 0.34/row
    #          of Pool SWDGE per op). grows*16B of descriptor ring per op
    #          must fit in swdge_scratch.
    #   "ind": legacy per-tile indirect_dma_start (128 rows/op, ~1us Pool
    #          SWDGE each). g_chunk: rows gathered per op per partition.
    #          KEEP AT 1: multi-row indirect DMA (3D dest AP) silently
    #          corrupts and can hard-wedge the device.
    # reps>1 repeats the main loop body (same data) for slope-timing in
    # test.py: device time scales with reps, dispatch overhead does not.
    f32 = mybir.dt.float32
    bf16 = mybir.dt.bfloat16
    i32 = mybir.dt.int32
    i16 = mybir.dt.int16
    Alu = mybir.AluOpType
    Act = mybir.ActivationFunctionType
    tdt = {"fp8": mybir.dt.float8e4, "bf16": bf16}[tab_dt]
    # R occupies 4 bytes (bf16 hi/lo pair) right after the C S-entries
    rslots = 4 // mybir.dt.size(tdt)
    TABW_ = tabw
    gpb = grows // P            # tiles covered per ant-gather op
    ng = NT // gpb              # ant-gather ops per pass
    assert NT % gpb == 0 and grows % P == 0
    if gs_bufs is None:
        # ng + 8: a full pass of gathered rows resident PLUS half a pass of
        # ring slack so a rep's gather ops don't WAR on the immediately
        # preceding pass's dots (which would expose the dot->Pool-gen->
        # transfer latency as a steady-state DMA stall)
        gs_bufs = ng + 8 if gather_mode == "ant" else 3

    nc = bacc.Bacc("TRN2", target_bir_lowering=False, debug=False,
                   dynamic_dma_scratch_size=swdge_scratch,
                   num_swdge_queues=swdge_queues)
    x_ap = nc.dram_tensor("x", [BLOC, C], f32, kind="ExternalInput").ap()
    ca_ap = nc.dram_tensor("ca", [C, C], f32, kind="ExternalInput").ap()
    if gather_mode == "ant":
        tgw_ap = nc.dram_tensor("tgw", [P, BLOC // 16], i16,
                                kind="ExternalInput").ap()
    else:
        tg_ap = nc.dram_tensor("tg", [BLOC], i32, kind="ExternalInput").ap()
    # out holds 3 per-sample grids [se | dot | r]; host computes
    # loss = ln(se)*r - dot (cheaper than a device-side tail: no Ln
    # activation-table load, no final DVE combine)
    out_ap = nc.dram_tensor("out", [P, 3 * NT], f32, kind="ExternalOutput").ap()
    tab_ap = nc.dram_tensor("tab", [C, TABW_], tdt).ap()

    with tile.TileContext(nc) as tc:
        with (
            tc.tile_pool(name="tabp", bufs=tabp_bufs) as tabp,
            tc.tile_pool(name="small", bufs=2) as small,
            tc.tile_pool(name="xs", bufs=xs_bufs) as xs,
            tc.tile_pool(name="gs", bufs=gs_bufs) as gs,
            tc.tile_pool(name="scr", bufs=scr_bufs) as scr,
            tc.tile_pool(name="cols", bufs=1) as cols,
        ):
            # target indices
            if gather_mode == "ant":
                idx16 = cols.tile([P, BLOC // 16], i16)
                nc.sync.dma_start(idx16[:], tgw_ap[:, :])
            else:
                idx = cols.tile([P, NT], i32)
                nc.sync.dma_start(idx[:], tg_ap.rearrange("(p c) -> p c", c=NT))

            # ---- smoothing table -------------------------------------------
            # SP issues DMAs in program order and blocks on each one's wait
            # sems, so: pre-issue the first tabp_bufs ca reads and the whole
            # x-prefetch window BEFORE the table loop (whose table-write DMAs
            # gate on compute), and prefetch later ca reads right after the
            # consuming exp. Otherwise the x pipeline can't start until the
            # last table write has been issued and DMA idles in the prologue.
            n_chunks = (C + P - 1) // P
            x_f = x_ap.rearrange("(p c) d -> p (c d)", c=NT)
            cat_tiles = {}

            def issue_ca(k):
                if k >= n_chunks:
                    return
                r0 = k * P
                pr = min(r0 + P, C) - r0
                t = tabp.tile([P, C], f32, tag="cat")
                nc.sync.dma_start(t[:pr], ca_ap[r0 : r0 + pr, :])
                cat_tiles[k] = t

            if pre_x is None:
                pre_x = xs_bufs
            ca_depth = tabp_bufs if ca_pre else 1
            for k in range(min(ca_depth, n_chunks)):
                issue_ca(k)
            xpre = {}
            if "x" not in abl:
                for c in range(min(pre_x, NT * reps // x_chunk)):
                    t = xs.tile([P, x_chunk * C], f32, tag="xbig")
                    nc.sync.dma_start(
                        t[:], x_f[:, c * x_chunk * C : (c + 1) * x_chunk * C]
                    )
                    xpre[c] = t

            for k in range(n_chunks):
                r0 = k * P
                pr = min(r0 + P, C) - r0
                cat = cat_tiles.pop(k)
                e = tabp.tile([P, C], f32, tag="e")
                sume = small.tile([P, 1], f32, tag="sume")
                nc.scalar.activation(e[:pr], cat[:pr], Act.Exp, accum_out=sume[:pr])
                issue_ca(k + ca_depth)
                # diagonal e[t, t] via affine mask + row reduce
                msk = tabp.tile([P, C], f32, tag="msk")
                nc.gpsimd.affine_select(
                    out=msk[:pr], in_=e[:pr], compare_op=Alu.is_equal, fill=0.0,
                    base=-r0, channel_multiplier=-1, pattern=[[1, C]],
                )
                ett = small.tile([P, 1], f32, tag="ett")
                nc.vector.tensor_reduce(
                    out=ett[:pr], in_=msk[:pr], axis=mybir.AxisListType.X, op=Alu.add
                )
                den = small.tile([P, 1], f32, tag="den")
                nc.vector.tensor_tensor(
                    out=den[:pr], in0=sume[:pr], in1=ett[:pr], op=Alu.subtract
                )
                rec = small.tile([P, 1], f32, tag="rec")
                nc.vector.reciprocal(rec[:pr], den[:pr])
                negw = small.tile([P, 1], f32, tag="negw")
                nc.vector.tensor_scalar_mul(negw[:pr], rec[:pr], -SM)
                # S_pre[t, j] = (e - sume) * (-s / den); its diagonal equals s,
                # and sum_j S_pre = R - (1 - 2s)
                spre = tabp.tile([P, C], f32, tag="spre")
                rpre = small.tile([P, 1], f32, tag="rpre")
                nc.vector.scalar_tensor_tensor(
                    out=spre[:pr], in0=e[:pr], scalar=sume[:pr],
                    in1=negw[:pr].to_broadcast([pr, C]),
                    op0=Alu.subtract, op1=Alu.mult, accum_out=rpre[:pr],
                )
                sb = tabp.tile([P, TABW_], tdt, tag="sb")
                nc.gpsimd.affine_select(
                    out=sb[:pr, 0:C], in_=spre[:pr], compare_op=Alu.not_equal,
                    fill=1.0 - SM, base=-r0, channel_multiplier=-1, pattern=[[1, C]],
                )
                # R as a hi/lo bf16 pair (R = hi + lo, error ~2^-18 relative),
                # bit-packed into the table row right after the S entries
                rt = small.tile([P, 1], f32, tag="rt")
                nc.vector.tensor_scalar_add(rt[:pr], rpre[:pr], 1.0 - 2 * SM)
                rv = sb[:pr, C : C + rslots].bitcast(bf16)
                nc.vector.tensor_copy(out=rv[:, 0:1], in_=rt[:pr])
                nc.vector.tensor_tensor(
                    out=rv[:, 1:2], in0=rt[:pr], in1=rv[:, 0:1], op=Alu.subtract
                )
                nc.vector.memset(sb[:pr, C + rslots : TABW_], 0.0)
                nc.sync.dma_start(tab_ap[r0 : r0 + pr, :], sb[:pr])

            # ---- main loop -------------------------------------------------
            # x viewed [P, NT*C]: partition p holds samples p*NT..p*NT+NT-1
            # back to back, so an x_chunk load is one contiguous 2D DMA
            x_r = x_ap.rearrange("(p c) d -> p c d", c=NT)
            outg = cols.tile([P, 3 * NT], f32)
            se_cols = outg[:, 0:NT]
            dot_cols = outg[:, NT : 2 * NT]
            r_cols = outg[:, 2 * NT : 3 * NT]
            if abl:
                for t in (se_cols, dot_cols, r_cols):
                    nc.vector.memset(t[:], 1.0)
            xt0 = gt0 = None
            if "x" in abl:
                xt0 = cols.tile([P, C], f32)
                nc.sync.dma_start(xt0[:], x_r[:, 0, :])
            if "gather" in abl:
                gt0 = cols.tile([P, TABW_], tdt)
                nc.vector.memset(gt0[:], 0.25)
            xbig = gbig = gant = None
            for jg in range(NT * reps):
                j = jg % NT
                if "x" in abl:
                    xt = xt0
                else:
                    if jg % x_chunk == 0:
                        c = jg // x_chunk
                        if c in xpre:
                            xbig = xpre.pop(c)
                        else:
                            xbig = xs.tile([P, x_chunk * C], f32)
                            nc.sync.dma_start(
                                xbig[:], x_f[:, j * C : (j + x_chunk) * C]
                            )
                    xt = xbig[:, (j % x_chunk) * C : (j % x_chunk + 1) * C]
                if "gather" in abl:
                    gt = gt0
                elif gather_mode == "ant":
                    if j % gpb == 0:
                        k = j // gpb
                        gant = gs.tile([P, gpb, TABW_], tdt, tag="gbig")
                        gcols = grows // 16
                        nc.gpsimd.dma_gather(
                            gant[:], tab_ap[:, :],
                            idx16[:, k * gcols : (k + 1) * gcols],
                            grows, grows, TABW_,
                        )
                        if "dve" not in abl:
                            # R for all gpb tiles of this op in one DVE op
                            grva = gant[:, :, C : C + rslots].bitcast(bf16)
                            nc.vector.tensor_tensor(
                                out=r_cols[:, j : j + gpb],
                                in0=grva[:, :, 0:1].rearrange("p g o -> p (g o)"),
                                in1=grva[:, :, 1:2].rearrange("p g o -> p (g o)"),
                                op=Alu.add,
                            )
                    gt = gant[:, j % gpb, :]
                elif g_chunk == 1:
                    gt = gs.tile([P, TABW_], tdt, tag="gbig")
                    nc.gpsimd.indirect_dma_start(
                        out=gt[:], out_offset=None, in_=tab_ap[:],
                        in_offset=bass.IndirectOffsetOnAxis(ap=idx[:, j : j + 1], axis=0),
                    )
                else:
                    if j % g_chunk == 0:
                        gbig = gs.tile([P, g_chunk, TABW_], tdt, tag="gbig")
                        nc.gpsimd.indirect_dma_start(
                            out=gbig[:], out_offset=None, in_=tab_ap[:],
                            in_offset=bass.IndirectOffsetOnAxis(
                                ap=idx[:, j : j + g_chunk], axis=0
                            ),
                        )
                    gt = gbig[:, j % g_chunk, :]
                if "act" not in abl:
                    es = scr.tile([P, C], bf16, tag="es")
                    nc.scalar.activation(
                        es[:], xt[:], Act.Exp, accum_out=se_cols[:, j : j + 1]
                    )
                if "dve" not in abl:
                    ps = scr.tile([P, C], f32, tag="ps")
                    nc.vector.scalar_tensor_tensor(
                        out=ps[:], in0=xt[:], scalar=1.0, in1=gt[:, 0:C],
                        op0=Alu.mult, op1=Alu.mult, accum_out=dot_cols[:, j : j + 1],
                    )
                    if gather_mode != "ant":
                        grv = gt[:, C : C + rslots].bitcast(bf16)
                        nc.vector.tensor_tensor(
                            out=r_cols[:, j : j + 1], in0=grv[:, 0:1],
                            in1=grv[:, 1:2], op=Alu.add,
                        )

            # ---- tail: one [se | dot | r] store; host does ln+combine ------
            nc.sync.dma_start(out_ap[:], outg[:])

    nc.compile()
    nc.finalize()
    return nc


def get_program():
    if "nc" not in _CACHE:
        _CACHE["nc"] = build_program()
    return _CACHE["nc"]


def wrap_targets(tg, grows=512):
    """Per-core targets [BLOC] -> DMAGatherAnt wrapped-int16 [128, BLOC/16].

    Op k (gathering `grows` rows) covers tiles k*gpb..k*gpb+gpb-1; flat
    position m of op k must hold the target of sample (m%128)*NT + k*gpb
    + m//128 so the gathered row lands at [partition m%128, block m//128].
    Wrapped layout: position m -> [m%16, k*(grows/16) + m//16], replicated
    across the 8 groups of 16 partitions.
    """
    gpb = grows // P
    ng = NT // gpb
    w = np.empty((16, BLOC // 16), dtype=np.int16)
    m = np.arange(grows)
    for k in range(ng):
        samples = (m % P) * NT + k * gpb + m // P
        w[m % 16, k * (grows // 16) + m // 16] = tg[samples]
    return np.tile(w, (8, 1))


def make_in_maps(x, class_avg, target):
    x = np.ascontiguousarray(np.asarray(x, dtype=np.float32))
    ca = np.ascontiguousarray(np.asarray(class_avg, dtype=np.float32))
    tg = np.ascontiguousarray(np.asarray(target).astype(np.int32))
    assert x.shape == (B, C) and ca.shape == (C, C) and tg.shape == (B,)
    return [
        {
            "x": x[c * BLOC : (c + 1) * BLOC],
            "ca": ca,
            "tg": tg[c * BLOC : (c + 1) * BLOC],
            "tgw": wrap_targets(tg[c * BLOC : (c + 1) * BLOC]),
        }
        for c in range(NCORES)
    ]


def per_sample_losses(out):
    """Device grids [P, 3*NT] -> per-sample losses [P, NT], float64."""
    out = out.astype(np.float64)
    se, dot, r = out[:, :NT], out[:, NT : 2 * NT], out[:, 2 * NT :]
    return np.log(se) * r - dot


def reduce_outputs(results):
    tot = 0.0
    for c in range(NCORES):
        tot += per_sample_losses(results[c]["out"]).sum()
    return np.array(tot / B, dtype=np.float32)


def kernel(x, class_avg, target):
    nc = get_program()
    in_maps = make_in_maps(x, class_avg, target)
    res = run_bass_kernel_spmd(nc, in_maps, list(range(NCORES)))
    return reduce_outputs(res.results)


# revision 49
# speedup vs baseline: 2.1220x; 1.6212x over previous
"""Class-conditional label-smoothing cross-entropy loss on 8 Trainium2 cores.

Reference math (C=1000 classes, B=65536 samples, smoothing s=0.1):
    A = softmax(class_avg, axis=-1)                         # [C, C]
    S[t, j] = s * (1 - A[t, j]) / (1 - A[t, t])  (j != t);  S[t, t] = 1 - s
    R[t]    = sum_j S[t, j]
    loss_i  = lse_i * R[t_i] - S[t_i] . x_i,   lse_i = log(sum_j exp(x_ij))
    out     = mean_i loss_i

Data-parallel: x and target are sharded along batch across the 8 cores,
class_avg is replicated. Each core:
  1. builds the fp8 smoothing table tab[t] = S[t, :] (1000 B + pad to 1024)
     in its DRAM once (fp8 quarters the per-sample gather traffic; since
     E[x]=0 the S quantization does not bias the mean loss)
  2. processes 64 tiles of 128 samples (sample p*64+j -> tile j, partition p):
     x tile DMA (x_chunk=2 tiles per 1 MiB transfer), per-tile indirect-DMA
     row gather of tab by target, ACT exp with accumulate -> sumexp, one
     fused DVE multiply with accumulate -> dot
  3. one [128, 2*64] store of the [sumexp | dot] grids. The host computes
     R[t] = s*(C-1)*sume[t]/den[t] + 1-2s in closed form (f64) and
     loss = ln(sumexp)*R[t] - dot, then the f64 mean over all cores.
     Host-side R + ln removes the per-tile DVE bf16-unpack, the table-build
     R bit-pack, and the device tail (Ln act-table load + final combine).

HW-measured notes (prep-order-symmetric ABBA A/B at reps=33, this host):
  - InstDMAGatherAnt (bulk 512-row gathers) is ~2.3x SLOWER than per-tile
    indirect_dma_start on real HW despite the cost model preferring it
    (+59us/pass measured); gather_mode="ind" is the default.
  - Buffer depths: xs=6, gs=6 optimal; deeper rings HURT on HW (xs=12
    +86us/pass, gs=16 +72) unlike TimelineSim which favors deep rings.
  - Timing is sensitive to SBUF pool base addresses (tabp 3->2 was
    -60us/pass with no steady-state code change).
  - Half-width (512 B) gather descriptors are +48us/pass: keep 1024 B rows.
  - x_chunk=4 is +59us/pass; 2 is best (matches prior session).
"""

import numpy as np

import concourse.bass as bass
import concourse.tile as tile
from concourse import bacc, mybir
from concourse.bass_utils import run_bass_kernel_spmd

B = 65536
C = 1000
NCORES = 8
BLOC = B // NCORES          # 8192 samples per core
P = 128
NT = BLOC // P              # 64 sample tiles per core
TABW = 1024                 # table row: 1000 fp8 S + f32 R (2 bf16 slots) + pad
SM = 0.1

_CACHE = {}


def build_program(reps=1, tab_dt="fp8", abl=(), x_chunk=2, g_chunk=1,
                  xs_bufs=6, gs_bufs=None, scr_bufs=2, tabp_bufs=2, tabw=TABW,
                  swdge_scratch=16384, swdge_queues=1,
                  gather_mode="ind", grows=512, pre_x=0, ca_pre=True,
                  x_layout="pmaj", host_r=True):
    # abl: timing-ablation switches ("gather" | "x" | "act" | "dve"), each
    # drops that component from the main loop (breaks numerics, timing only).
    # x_chunk: sample tiles per x DMA (2 -> 1 MiB transfers; HW-measured
    # faster than 1 or 8).
    # gather_mode:
    #   "ant": InstDMAGatherAnt, `grows` table rows per op (994ns + 0.34/row
    #          of Pool SWDGE per op). grows*16B of descriptor ring per op
    #          must fit in swdge_scratch.
    #   "ind": legacy per-tile indirect_dma_start (128 rows/op, ~1us Pool
    #          SWDGE each). g_chunk: rows gathered per op per partition.
    #          KEEP AT 1: multi-row indirect DMA (3D dest AP) silently
    #          corrupts and can hard-wedge the device.
    # reps>1 repeats the main loop body (same data) for slope-timing in
    # test.py: device time scales with reps, dispatch overhead does not.
    f32 = mybir.dt.float32
    bf16 = mybir.dt.bfloat16
    i32 = mybir.dt.int32
    i16 = mybir.dt.int16
    Alu = mybir.AluOpType
    Act = mybir.ActivationFunctionType
    tdt = {"fp8": mybir.dt.float8e4, "bf16": bf16}[tab_dt]
    # R occupies 4 bytes (bf16 hi/lo pair) right after the C S-entries
    rslots = 4 // mybir.dt.size(tdt)
    TABW_ = tabw
    gpb = grows // P            # tiles covered per ant-gather op
    ng = NT // gpb              # ant-gather ops per pass
    assert NT % gpb == 0 and grows % P == 0
    if gs_bufs is None:
        # ant: ng + 8 = a full pass of gathered rows resident PLUS half a
        # pass of ring slack so a rep's gather ops don't WAR on the
        # immediately preceding pass's dots. ind: 6 HW-measured best via
        # prep-order-symmetric (ABBA) A/B: 4 is +29us/pass, 8 +7, 16 +72.
        # NOTE HW timing is also sensitive to SBUF pool placement (pool
        # sizes shift later pools' base addresses; tabp 3->2 alone was
        # -60us/pass), so buffer counts here encode address luck too.
        gs_bufs = ng + 8 if gather_mode == "ant" else 6

    nc = bacc.Bacc("TRN2", target_bir_lowering=False, debug=False,
                   dynamic_dma_scratch_size=swdge_scratch,
                   num_swdge_queues=swdge_queues)
    x_ap = nc.dram_tensor("x", [BLOC, C], f32, kind="ExternalInput").ap()
    ca_ap = nc.dram_tensor("ca", [C, C], f32, kind="ExternalInput").ap()
    if gather_mode == "ant":
        tgw_ap = nc.dram_tensor("tgw", [P, BLOC // 16], i16,
                                kind="ExternalInput").ap()
    elif x_layout == "cmaj":
        tgc_ap = nc.dram_tensor("tgc", [P, NT], i32, kind="ExternalInput").ap()
    else:
        tg_ap = nc.dram_tensor("tg", [BLOC], i32, kind="ExternalInput").ap()
    # out holds per-sample grids [se | dot] (+ [r] when not host_r); host
    # computes loss = ln(se)*R[t] - dot (cheaper than a device-side tail: no
    # Ln activation-table load, no final DVE combine). With host_r the R
    # table itself comes from a closed-form host computation
    # (R[t] = s*(C-1)*sume[t]/den[t] + 1 - 2s), dropping the per-tile DVE
    # bf16-unpack ops and the table-build R bit-pack entirely.
    ngrids = 2 if host_r else 3
    out_ap = nc.dram_tensor("out", [P, ngrids * NT], f32,
                            kind="ExternalOutput").ap()
    tab_ap = nc.dram_tensor("tab", [C, TABW_], tdt).ap()

    with tile.TileContext(nc) as tc:
        with (
            tc.tile_pool(name="tabp", bufs=tabp_bufs) as tabp,
            tc.tile_pool(name="small", bufs=2) as small,
            tc.tile_pool(name="xs", bufs=xs_bufs) as xs,
            tc.tile_pool(name="gs", bufs=gs_bufs) as gs,
            tc.tile_pool(name="scr", bufs=scr_bufs) as scr,
            tc.tile_pool(name="cols", bufs=1) as cols,
        ):
            # target indices
            if gather_mode == "ant":
                idx16 = cols.tile([P, BLOC // 16], i16)
                nc.sync.dma_start(idx16[:], tgw_ap[:, :])
            elif x_layout == "cmaj":
                idx = cols.tile([P, NT], i32)
                nc.sync.dma_start(idx[:], tgc_ap[:, :])
            else:
                idx = cols.tile([P, NT], i32)
                nc.sync.dma_start(idx[:], tg_ap.rearrange("(p c) -> p c", c=NT))

            # ---- smoothing table -------------------------------------------
            # SP issues DMAs in program order and blocks on each one's wait
            # sems, so: pre-issue the first tabp_bufs ca reads and the whole
            # x-prefetch window BEFORE the table loop (whose table-write DMAs
            # gate on compute), and prefetch later ca reads right after the
            # consuming exp. Otherwise the x pipeline can't start until the
            # last table write has been issued and DMA idles in the prologue.
            n_chunks = (C + P - 1) // P
            # pmaj: partition p holds samples p*NT..p*NT+NT-1 (contiguous per
            # partition, 8 KB descriptors at 256 KB stride). cmaj: tile j holds
            # samples j*128..j*128+127 (each chunk DMA reads one contiguous
            # 512 KB * x_chunk DRAM span in 4 KB descriptors).
            x_f = x_ap.rearrange("(p c) d -> p (c d)", c=NT)
            x_c3 = x_ap.rearrange("(c p) d -> p c d", p=P)

            def x_dma(xbig, j):
                if "xh" in abl:
                    # timing probe: load only half the x bytes per chunk
                    nc.sync.dma_start(
                        xbig[:, : x_chunk * C // 2],
                        x_f[:, j * C : j * C + x_chunk * C // 2],
                    )
                elif x_layout == "cmaj":
                    nc.sync.dma_start(
                        xbig.rearrange("p (a c) -> p a c", c=C),
                        x_c3[:, j : j + x_chunk, :],
                    )
                else:
                    nc.sync.dma_start(xbig[:], x_f[:, j * C : (j + x_chunk) * C])

            cat_tiles = {}

            def issue_ca(k):
                if k >= n_chunks:
                    return
                r0 = k * P
                pr = min(r0 + P, C) - r0
                t = tabp.tile([P, C], f32, tag="cat")
                nc.sync.dma_start(t[:pr], ca_ap[r0 : r0 + pr, :])
                cat_tiles[k] = t

            if pre_x is None:
                pre_x = xs_bufs
            ca_depth = tabp_bufs if ca_pre else 1
            for k in range(min(ca_depth, n_chunks)):
                issue_ca(k)
            xpre = {}
            if "x" not in abl:
                for c in range(min(pre_x, NT * reps // x_chunk)):
                    t = xs.tile([P, x_chunk * C], f32, tag="xbig")
                    x_dma(t, c * x_chunk)
                    xpre[c] = t

            for k in range(n_chunks):
                r0 = k * P
                pr = min(r0 + P, C) - r0
                cat = cat_tiles.pop(k)
                e = tabp.tile([P, C], f32, tag="e")
                sume = small.tile([P, 1], f32, tag="sume")
                nc.scalar.activation(e[:pr], cat[:pr], Act.Exp, accum_out=sume[:pr])
                issue_ca(k + ca_depth)
                # diagonal e[t, t] via affine mask + row reduce
                msk = tabp.tile([P, C], f32, tag="msk")
                nc.gpsimd.affine_select(
                    out=msk[:pr], in_=e[:pr], compare_op=Alu.is_equal, fill=0.0,
                    base=-r0, channel_multiplier=-1, pattern=[[1, C]],
                )
                ett = small.tile([P, 1], f32, tag="ett")
                nc.vector.tensor_reduce(
                    out=ett[:pr], in_=msk[:pr], axis=mybir.AxisListType.X, op=Alu.add
                )
                den = small.tile([P, 1], f32, tag="den")
                nc.vector.tensor_tensor(
                    out=den[:pr], in0=sume[:pr], in1=ett[:pr], op=Alu.subtract
                )
                rec = small.tile([P, 1], f32, tag="rec")
                nc.vector.reciprocal(rec[:pr], den[:pr])
                negw = small.tile([P, 1], f32, tag="negw")
                nc.vector.tensor_scalar_mul(negw[:pr], rec[:pr], -SM)
                # S_pre[t, j] = (e - sume) * (-s / den); its diagonal equals s,
                # and sum_j S_pre = R - (1 - 2s)
                spre = tabp.tile([P, C], f32, tag="spre")
                rpre = small.tile([P, 1], f32, tag="rpre")
                nc.vector.scalar_tensor_tensor(
                    out=spre[:pr], in0=e[:pr], scalar=sume[:pr],
                    in1=negw[:pr].to_broadcast([pr, C]),
                    op0=Alu.subtract, op1=Alu.mult, accum_out=rpre[:pr],
                )
                sb = tabp.tile([P, TABW_], tdt, tag="sb")
                nc.gpsimd.affine_select(
                    out=sb[:pr, 0:C], in_=spre[:pr], compare_op=Alu.not_equal,
                    fill=1.0 - SM, base=-r0, channel_multiplier=-1, pattern=[[1, C]],
                )
                if host_r:
                    nc.vector.memset(sb[:pr, C:TABW_], 0.0)
                else:
                    # R as a hi/lo bf16 pair (R = hi + lo, error ~2^-18
                    # relative), bit-packed right after the S entries
                    rt = small.tile([P, 1], f32, tag="rt")
                    nc.vector.tensor_scalar_add(rt[:pr], rpre[:pr], 1.0 - 2 * SM)
                    rv = sb[:pr, C : C + rslots].bitcast(bf16)
                    nc.vector.tensor_copy(out=rv[:, 0:1], in_=rt[:pr])
                    nc.vector.tensor_tensor(
                        out=rv[:, 1:2], in0=rt[:pr], in1=rv[:, 0:1], op=Alu.subtract
                    )
                    nc.vector.memset(sb[:pr, C + rslots : TABW_], 0.0)
                nc.sync.dma_start(tab_ap[r0 : r0 + pr, :], sb[:pr])

            # ---- main loop -------------------------------------------------
            # x viewed [P, NT*C]: partition p holds samples p*NT..p*NT+NT-1
            # back to back, so an x_chunk load is one contiguous 2D DMA
            x_r = x_ap.rearrange("(p c) d -> p c d", c=NT)
            outg = cols.tile([P, ngrids * NT], f32)
            se_cols = outg[:, 0:NT]
            dot_cols = outg[:, NT : 2 * NT]
            r_cols = None if host_r else outg[:, 2 * NT : 3 * NT]
            if abl:
                nc.vector.memset(outg[:], 1.0)
            xt0 = gt0 = None
            if "x" in abl:
                xt0 = cols.tile([P, C], f32)
                nc.sync.dma_start(xt0[:], x_r[:, 0, :])
            if "gather" in abl:
                gt0 = cols.tile([P, TABW_], tdt)
                nc.vector.memset(gt0[:], 0.25)
            xbig = gbig = gant = None
            for jg in range(NT * reps):
                j = jg % NT
                if "x" in abl:
                    xt = xt0
                else:
                    if jg % x_chunk == 0:
                        c = jg // x_chunk
                        if c in xpre:
                            xbig = xpre.pop(c)
                        else:
                            xbig = xs.tile([P, x_chunk * C], f32, tag="xbig")
                            x_dma(xbig, j)
                    xt = xbig[:, (j % x_chunk) * C : (j % x_chunk + 1) * C]
                if "gather" in abl:
                    gt = gt0
                elif gather_mode == "ant":
                    if j % gpb == 0:
                        k = j // gpb
                        gant = gs.tile([P, gpb, TABW_], tdt, tag="gbig")
                        gcols = grows // 16
                        nc.gpsimd.dma_gather(
                            gant[:], tab_ap[:, :],
                            idx16[:, k * gcols : (k + 1) * gcols],
                            grows, grows, TABW_,
                        )
                        if "dve" not in abl and not host_r:
                            # R for all gpb tiles of this op in one DVE op
                            grva = gant[:, :, C : C + rslots].bitcast(bf16)
                            nc.vector.tensor_tensor(
                                out=r_cols[:, j : j + gpb],
                                in0=grva[:, :, 0:1].rearrange("p g o -> p (g o)"),
                                in1=grva[:, :, 1:2].rearrange("p g o -> p (g o)"),
                                op=Alu.add,
                            )
                    gt = gant[:, j % gpb, :]
                elif g_chunk == 1:
                    gt = gs.tile([P, TABW_], tdt, tag="gbig")
                    gw = TABW_ // 2 if "gh" in abl else TABW_
                    nc.gpsimd.indirect_dma_start(
                        out=gt[:, :gw], out_offset=None, in_=tab_ap[:, :gw],
                        in_offset=bass.IndirectOffsetOnAxis(ap=idx[:, j : j + 1], axis=0),
                    )
                else:
                    if j % g_chunk == 0:
                        gbig = gs.tile([P, g_chunk, TABW_], tdt, tag="gbig")
                        nc.gpsimd.indirect_dma_start(
                            out=gbig[:], out_offset=None, in_=tab_ap[:],
                            in_offset=bass.IndirectOffsetOnAxis(
                                ap=idx[:, j : j + g_chunk], axis=0
                            ),
                        )
                    gt = gbig[:, j % g_chunk, :]
                if "act" not in abl:
                    es = scr.tile([P, C], bf16, tag="es")
                    nc.scalar.activation(
                        es[:], xt[:], Act.Exp, accum_out=se_cols[:, j : j + 1]
                    )
                if "dve" not in abl:
                    ps = scr.tile([P, C], f32, tag="ps")
                    nc.vector.scalar_tensor_tensor(
                        out=ps[:], in0=xt[:], scalar=1.0, in1=gt[:, 0:C],
                        op0=Alu.mult, op1=Alu.mult, accum_out=dot_cols[:, j : j + 1],
                    )
                    if gather_mode != "ant" and not host_r:
                        grv = gt[:, C : C + rslots].bitcast(bf16)
                        nc.vector.tensor_tensor(
                            out=r_cols[:, j : j + 1], in0=grv[:, 0:1],
                            in1=grv[:, 1:2], op=Alu.add,
                        )

            # ---- tail: one [se | dot | r] store; host does ln+combine ------
            nc.sync.dma_start(out_ap[:], outg[:])

    nc.compile()
    nc.finalize()
    return nc


def get_program():
    if "nc" not in _CACHE:
        _CACHE["nc"] = build_program()
    return _CACHE["nc"]


def wrap_targets(tg, grows=512):
    """Per-core targets [BLOC] -> DMAGatherAnt wrapped-int16 [128, BLOC/16].

    Op k (gathering `grows` rows) covers tiles k*gpb..k*gpb+gpb-1; flat
    position m of op k must hold the target of sample (m%128)*NT + k*gpb
    + m//128 so the gathered row lands at [partition m%128, block m//128].
    Wrapped layout: position m -> [m%16, k*(grows/16) + m//16], replicated
    across the 8 groups of 16 partitions.
    """
    gpb = grows // P
    ng = NT // gpb
    w = np.empty((16, BLOC // 16), dtype=np.int16)
    m = np.arange(grows)
    for k in range(ng):
        samples = (m % P) * NT + k * gpb + m // P
        w[m % 16, k * (grows // 16) + m // 16] = tg[samples]
    return np.tile(w, (8, 1))


def make_in_maps(x, class_avg, target):
    x = np.ascontiguousarray(np.asarray(x, dtype=np.float32))
    ca = np.ascontiguousarray(np.asarray(class_avg, dtype=np.float32))
    tg = np.ascontiguousarray(np.asarray(target).astype(np.int32))
    assert x.shape == (B, C) and ca.shape == (C, C) and tg.shape == (B,)
    return [
        {
            "x": x[c * BLOC : (c + 1) * BLOC],
            "ca": ca,
            "tg": tg[c * BLOC : (c + 1) * BLOC],
            "tgw": wrap_targets(tg[c * BLOC : (c + 1) * BLOC]),
            # cmaj layout: idx[p, j] = target of sample j*128+p
            "tgc": np.ascontiguousarray(
                tg[c * BLOC : (c + 1) * BLOC].reshape(NT, P).T
            ),
        }
        for c in range(NCORES)
    ]


def host_r_table(class_avg):
    """R[t] = sum_j S[t, j] in closed form, float64.

    R[t] = s*(C-1)*sume[t]/den[t] + 1 - 2s with sume = sum_j exp(ca[t, :]),
    den = sume - exp(ca[t, t]).
    """
    a = np.exp(class_avg.astype(np.float64))
    sume = a.sum(axis=1)
    den = sume - np.diagonal(a)
    return SM * (C - 1) * sume / den + 1.0 - 2 * SM


def per_sample_losses(out, rgrid=None):
    """Device grids [P, ngrids*NT] -> per-sample losses [P, NT], float64.

    rgrid: per-sample R values [P, NT] (host_r mode); if None, R is read
    from the third device grid.
    """
    out = out.astype(np.float64)
    se, dot = out[:, :NT], out[:, NT : 2 * NT]
    if rgrid is None:
        rgrid = out[:, 2 * NT : 3 * NT]
    return np.log(se) * rgrid - dot


def reduce_outputs(results, rtab=None, tg=None):
    tot = 0.0
    for c in range(NCORES):
        rgrid = None
        if rtab is not None:
            # out[p, j] = sample p*NT + j of this core's shard
            rgrid = rtab[tg[c * BLOC : (c + 1) * BLOC]].reshape(P, NT)
        tot += per_sample_losses(results[c]["out"], rgrid).sum()
    return np.array(tot / B, dtype=np.float32)


def kernel(x, class_avg, target):
    nc = get_program()
    in_maps = make_in_maps(x, class_avg, target)
    res = run_bass_kernel_spmd(nc, in_maps, list(range(NCORES)))
    tg = np.asarray(target).astype(np.int64)
    return reduce_outputs(res.results, host_r_table(np.asarray(class_avg)), tg)
